# revision 4
# baseline (speedup 1.0000x reference)
"""Trainium2 Bass kernel for nn_DecoderOnlyTransformer_10041633538673 (v3).

Sharding: core c = (batch c//4, rank r=c%4) owns heads 4r..4r+3 of its
batch. Head-sharded QKV+attention; one 8-core AllToAll per head-pair
re-shards to sequence-split; Wo/l2norm/Wff/l2norm/gelu on the local
512-query slice.

v3 performance structure:
    - bf16 x/Wq/Wk/Wv (halves startup DMA); per-dc weight DMAs so the
      first projection matmul starts as soon as chunk 0 lands.
    - pair-0 q/k projected first on its own 4 PSUM banks (freed before
      attention); V and pair-1 q/k cycle through the attn-out banks so
      they overlap pair-0's exp stream.
    - attention is Scalar(exp)-paced; matmuls grouped per head (shared
      stationary operand) with walrus ldw-opt eliding redundant weight
      loads; softmax Z via ACT accum_out (cheaper than a DVE reduce).
    - outgoing AllToAll blocks are multiplied by a per-core 0/1 batch
      mask (reads attn-out straight from PSUM, no staging copy);
      receivers add block j + j+4 (exact: one side is zeros), halving
      the Wo contraction to the 4 same-batch blocks.
    - Wo's t=0 half runs inside the second AllToAll's wait window; t=1
      accumulates on top via a DVE add. Gelu/square read Wff's PSUM
      output directly.
"""

import os
import numpy as np

import concourse.bass as bass
import concourse.tile as tile
from concourse import bacc, mybir
from concourse.bass_utils import run_bass_kernel_spmd
from concourse import bass_utils as _bass_utils
from concourse.masks import make_identity

if os.environ.get("KERNEL_LDW_OPT", "0") == "1" and not getattr(_bass_utils, "_ldw_opt_patched", False):
    # consecutive matmuls sharing a stationary operand: walrus's ldw-opt
    # pass elides the redundant weight reloads
    _orig_run_command = _bass_utils.run_command

    def _run_command_ldw(cmd, **kw):
        cmd = [
            "--enable-ldw-opt=true" if c == "--enable-ldw-opt=false" else c
            for c in cmd
        ]
        return _orig_run_command(cmd, **kw)

    _bass_utils.run_command = _run_command_ldw
    _bass_utils._ldw_opt_patched = True

F32 = mybir.dt.float32
F32R = mybir.dt.float32r
BF16 = mybir.dt.bfloat16

P = 128
S = 2048
D = 1024
DH = 64
LC = 256        # local head-cols per core (4 heads x 64)
DC = D // P     # 8 contraction chunks
SBLK = S // P   # 16 seq blocks
SLICE = S // 4  # 512-query slice per core

AF = mybir.ActivationFunctionType
ALU = mybir.AluOpType

SIM_NO_GELU = os.environ.get("KERNEL_SIM_NO_GELU", "0") == "1"


def build_program():
    nc = bacc.Bacc(
        "TRN2",
        target_bir_lowering=False,
        debug=False,
        enable_asserts=False,
        num_devices=8,
    )

    xT = nc.dram_tensor("xT", [D, S], BF16, kind="ExternalInput").ap()
    wq = nc.dram_tensor("wq", [D, LC], BF16, kind="ExternalInput").ap()
    wk = nc.dram_tensor("wk", [D, LC], BF16, kind="ExternalInput").ap()
    wv = nc.dram_tensor("wv", [D, LC], BF16, kind="ExternalInput").ap()
    wo2 = nc.dram_tensor("wo2", [2, 4 * P, D], BF16, kind="ExternalInput").ap()
    wff = nc.dram_tensor("wff", [D, D], F32R, kind="ExternalInput").ap()
    msk = nc.dram_tensor("msk", [P, 8], F32, kind="ExternalInput").ap()
    out = nc.dram_tensor("out", [SLICE, D], F32, kind="ExternalOutput").ap()

    cc_in = [
        nc.dram_tensor(f"cc_in{t}", [8 * P, SLICE], BF16).ap() for t in range(2)
    ]
    cc_out = [
        nc.dram_tensor(f"cc_out{t}", [8 * P, SLICE], BF16).ap() for t in range(2)
    ]

    with tile.TileContext(nc) as tc:
        misc = tc.alloc_tile_pool(name="misc", bufs=1)
        ident = misc.tile([P, P], F32)
        make_identity(nc, ident)
        msk_sb = misc.tile([P, 8], F32, tag="msk")
        nc.sync.dma_start(out=msk_sb, in_=msk)

        qkv = tc.alloc_tile_pool(name="qkv", bufs=1)
        qt = [qkv.tile([P, S], BF16, tag=f"qt{t}", name=f"qt{t}") for t in range(2)]
        kt = [qkv.tile([P, S], BF16, tag=f"kt{t}", name=f"kt{t}") for t in range(2)]
        v_sb = qkv.tile([P, SBLK, LC], BF16, tag="v")

        # weights for the post-attention phases (DMA overlaps phase 1)
        w2 = tc.alloc_tile_pool(name="w2", bufs=1)
        wo2_sb = w2.tile([P, 2, 4, D], BF16, tag="wo2")
        nc.sync.dma_start(out=wo2_sb, in_=wo2.rearrange("t (i p) d -> p t i d", p=P))
        st = w2.tile([P, 2, 8, SLICE], BF16, tag="st")
        stp = w2.tile([P, 2, 4, SLICE], BF16, tag="stp")

        # attn-out PSUM banks (also cycle V + pair-1 q/k projections);
        # closes before the tail so its banks return for Wff
        opp_cm = tc.tile_pool(name="op", bufs=1, space="PSUM")
        opp = opp_cm.__enter__()

        # ---- Phase 1: x^T + projection weights; Q^T/K^T/V.
        with tc.tile_pool(name="xtw", bufs=1) as xtw, tc.tile_pool(
            name="pp1", bufs=1, space="PSUM"
        ) as pp1:
            wq_sb = xtw.tile([P, DC, LC], BF16, tag="wq")
            wk_sb = xtw.tile([P, DC, LC], BF16, tag="wk")
            wv_sb = xtw.tile([P, DC, LC], BF16, tag="wv")
            xt = xtw.tile([P, DC, S], BF16, tag="xt")
            for dc in range(DC):
                nc.sync.dma_start(
                    out=wq_sb[:, dc, :], in_=wq[dc * P : (dc + 1) * P, :]
                )
                nc.sync.dma_start(
                    out=wk_sb[:, dc, :], in_=wk[dc * P : (dc + 1) * P, :]
                )
                nc.sync.dma_start(out=xt[:, dc, :], in_=xT[dc * P : (dc + 1) * P, :])
            nc.sync.dma_start(out=wv_sb, in_=wv.rearrange("(c p) m -> p c m", p=P))

            # pair-0 q/k on dedicated banks (pj*), freed before attention
            for w_sb, dst in ((wq_sb, qt), (wk_sb, kt)):
                pst = [
                    pp1.tile([P, 512], F32, tag=f"pj{i}", name=f"pj{i}")
                    for i in range(4)
                ]
                for dc in range(DC):
                    for sb in range(4):
                        nc.tensor.matmul(
                            pst[sb],
                            lhsT=w_sb[:, dc, 0:P],
                            rhs=xt[:, dc, sb * 512 : (sb + 1) * 512],
                            start=(dc == 0),
                            stop=(dc == DC - 1),
                        )
                for sb in range(4):
                    nc.vector.tensor_copy(
                        out=dst[0][:, sb * 512 : (sb + 1) * 512], in_=pst[sb]
                    )
            # V, then pair-1 q/k, cycling through the op banks
            for sb in range(SBLK):
                ps = opp.tile([P, 512], F32, tag=f"op{sb % 4}", name=f"pv{sb}")
                for dc in range(DC):
                    nc.tensor.matmul(
                        ps[:, 0:LC],
                        lhsT=xt[:, dc, sb * P : (sb + 1) * P],
                        rhs=wv_sb[:, dc, :],
                        start=(dc == 0),
                        stop=(dc == DC - 1),
                    )
                nc.vector.tensor_copy(out=v_sb[:, sb, :], in_=ps[:, 0:LC])
            for w_sb, dst in ((wq_sb, qt), (wk_sb, kt)):
                pst = [
                    opp.tile([P, 512], F32, tag=f"op{i}", name=f"pj1{i}")
                    for i in range(4)
                ]
                for dc in range(DC):
                    for sb in range(4):
                        nc.tensor.matmul(
                            pst[sb],
                            lhsT=w_sb[:, dc, P : 2 * P],
                            rhs=xt[:, dc, sb * 512 : (sb + 1) * 512],
                            start=(dc == 0),
                            stop=(dc == DC - 1),
                        )
                for sb in range(4):
                    nc.vector.tensor_copy(
                        out=dst[1][:, sb * 512 : (sb + 1) * 512], in_=pst[sb]
                    )

        # ---- Phase 2: attention. E = exp(scores/32); Z per head via ACT
        # accum; 1/Z folded into V rows; out^T accumulated in PSUM with
        # the 2 heads of a pair packed into PE column strips.
        with tc.tile_pool(name="att", bufs=10) as att, tc.tile_pool(
            name="sc", bufs=2, space="PSUM"
        ) as scp, tc.tile_pool(name="asml", bufs=4) as asml, tc.tile_pool(
            name="mcp", bufs=2
        ) as mcp:
            for t in range(2):
                o_pp = [
                    opp.tile([P, 512], F32, tag=f"op{qb}", name=f"op{t}{qb}")
                    for qb in range(4)
                ]
                for kb in range(SBLK):
                    kb0 = kb * P
                    e_a = att.tile([P, S], BF16, tag="e", name="e_a")
                    e_b = att.tile([P, S], BF16, tag="e", name="e_b")
                    zp_a = asml.tile([P, 2], F32, tag="zp", name="zp_a")
                    zp_b = asml.tile([P, 2], F32, tag="zp", name="zp_b")
                    # per head: 4 score MMs sharing one stationary kt strip
                    # (ldw-opt elides 3 reloads), then the 2 exps
                    for hh, e_t, zp in ((0, e_a, zp_a), (1, e_b, zp_b)):
                        r0 = hh * DH
                        scs = []
                        for qh in range(2):
                            sc = scp.tile([P, 1024], F32, tag="sc", name=f"sc{hh}{qh}")
                            scs.append(sc)
                            for qs in range(2):
                                q0 = qh * 1024 + qs * 512
                                nc.tensor.matmul(
                                    sc[:, qs * 512 : (qs + 1) * 512],
                                    lhsT=kt[t][r0 : r0 + DH, kb0 : kb0 + P],
                                    rhs=qt[t][r0 : r0 + DH, q0 : q0 + 512],
                                    start=True,
                                    stop=True,
                                    tile_position=(r0, 0),
                                    skip_group_check=True,
                                )
                        for qh in range(2):
                            nc.scalar.activation(
                                out=e_t[:, qh * 1024 : (qh + 1) * 1024],
                                in_=scs[qh],
                                func=AF.Exp,
                                scale=1.0 / 32.0,
                                accum_out=zp[:, qh : qh + 1],
                            )
                    # fold 1/Z into the V rows, then all 8 attn@V matmuls,
                    # grouped per head (shared stationary vp)
                    vps = []
                    for hh, zp in ((0, zp_a), (1, zp_b)):
                        zs = asml.tile([P, 1], F32, tag="zs", name="zs")
                        nc.vector.tensor_add(out=zs, in0=zp[:, 0:1], in1=zp[:, 1:2])
                        zr = asml.tile([P, 1], F32, tag="zr", name="zr")
                        nc.vector.reciprocal(out=zr, in_=zs)
                        vp = asml.tile([P, DH], BF16, tag="vp", name="vp")
                        nc.vector.tensor_scalar_mul(
                            out=vp,
                            in0=v_sb[:, kb, (2 * t + hh) * DH : (2 * t + hh + 1) * DH],
                            scalar1=zr,
                        )
                        vps.append(vp)
                    for hh, e_t in ((0, e_a), (1, e_b)):
                        for qb in range(4):
                            nc.tensor.matmul(
                                o_pp[qb][hh * DH : (hh + 1) * DH, :],
                                lhsT=vps[hh],
                                rhs=e_t[:, qb * 512 : (qb + 1) * 512],
                                start=(kb == 0),
                                stop=(kb == SBLK - 1),
                                tile_position=(0, hh * DH),
                                skip_group_check=True,
                            )
                # stage the AllToAll: mask out cross-batch destination
                # blocks (reads o_pp straight from PSUM)
                mcc = mcp.tile([P, 8, SLICE], BF16, tag="mcc", name=f"mcc{t}")
                for j in range(8):
                    nc.vector.tensor_scalar_mul(
                        out=mcc[:, j, :],
                        in0=o_pp[j % 4],
                        scalar1=msk_sb[:, j : j + 1],
                    )
                    nc.sync.dma_start(
                        out=cc_in[t][j * P : (j + 1) * P, :], in_=mcc[:, j, :]
                    )
                nc.gpsimd.collective_compute(
                    "AllToAll",
                    ALU.bypass,
                    replica_groups=[[0, 1, 2, 3, 4, 5, 6, 7]],
                    ins=[cc_in[t]],
                    outs=[cc_out[t]],
                )

        opp_cm.__exit__(None, None, None)

        # ---- Phase 3: Wo (t=0 half early), l2norm, Wff, l2norm, gelu
        with tc.tile_pool(name="pp2", bufs=2, space="PSUM") as pp2, tc.tile_pool(
            name="tail", bufs=2) as tl, tc.tile_pool(
            name="tsm", bufs=1
        ) as tsm:
            wff_sb = tsm.tile([P, DC, D], F32R, tag="wff")
            nc.sync.dma_start(out=wff_sb, in_=wff.rearrange("(c p) d -> p c d", p=P))
            for t in range(2):
                for i in range(8):
                    nc.sync.dma_start(
                        out=st[:, t, i, :], in_=cc_out[t][i * P : (i + 1) * P, :]
                    )
                for i in range(4):
                    # cross-batch side is exactly zero -> add is exact
                    nc.vector.tensor_add(
                        out=stp[:, t, i, :],
                        in0=st[:, t, i, :],
                        in1=st[:, t, i + 4, :],
                    )

            # t=0 half of Wo runs inside the second AllToAll's wait window
            z0 = tl.tile([P, 4, D], F32, tag="z0")
            for sb in range(4):
                for db in range(2):
                    ps = pp2.tile([P, 512], F32, tag="pp2")
                    for i in range(4):
                        nc.tensor.matmul(
                            ps,
                            lhsT=stp[:, 0, i, sb * P : (sb + 1) * P],
                            rhs=wo2_sb[:, 0, i, db * 512 : (db + 1) * 512],
                            start=(i == 0),
                            stop=(i == 3),
                        )
                    nc.vector.tensor_copy(
                        out=z0[:, sb, db * 512 : (db + 1) * 512], in_=ps
                    )

            z_sb = tl.tile([P, 4, D], F32, tag="big")
            for sb in range(4):
                for db in range(2):
                    ps = pp2.tile([P, 512], F32, tag="pp2")
                    for i in range(4):
                        nc.tensor.matmul(
                            ps,
                            lhsT=stp[:, 1, i, sb * P : (sb + 1) * P],
                            rhs=wo2_sb[:, 1, i, db * 512 : (db + 1) * 512],
                            start=(i == 0),
                            stop=(i == 3),
                        )
                    nc.vector.tensor_add(
                        out=z_sb[:, sb, db * 512 : (db + 1) * 512],
                        in0=ps,
                        in1=z0[:, sb, db * 512 : (db + 1) * 512],
                    )

            junk = tsm.tile([P, D], F32, tag="junk")
            ss1 = tsm.tile([P, 4], F32, tag="ss1")
            rs1 = tsm.tile([P, 4], F32, tag="rs1")
            for sb in range(4):
                nc.scalar.activation(
                    out=junk,
                    in_=z_sb[:, sb, :],
                    func=AF.Square,
                    accum_out=ss1[:, sb : sb + 1],
                )
            # 1/sqrt(ss) = exp(-0.5 * ln(ss)); Ln+Exp share one ACT table set
            nc.scalar.activation(out=ss1, in_=ss1, func=AF.Ln)
            nc.scalar.activation(out=rs1, in_=ss1, func=AF.Exp, scale=-0.5)
            for sb in range(4):
                nc.vector.tensor_scalar_mul(
                    out=z_sb[:, sb, :], in0=z_sb[:, sb, :], scalar1=rs1[:, sb : sb + 1]
                )

            # n1^T via PE transpose
            n1t = tsm.tile([P, DC, SLICE], F32R, tag="n1t")
            for sb in range(4):
                for dc in range(DC):
                    tp = pp2.tile([P, P], F32, tag="tp")
                    nc.tensor.transpose(
                        tp, z_sb[:, sb, dc * P : (dc + 1) * P], ident
                    )
                    nc.vector.tensor_copy(
                        out=n1t[:, dc, sb * P : (sb + 1) * P], in_=tp
                    )

            y_sb = tl.tile([P, 4, D], F32, tag="big")
            ss2 = tsm.tile([P, 4], F32, tag="ss2")
            rs2 = tsm.tile([P, 4], F32, tag="rs2")
            for sb in range(4):
                for db in range(2):
                    ps = pp2.tile([P, 512], F32, tag="pp2")
                    for dc in range(DC):
                        nc.tensor.matmul(
                            ps,
                            lhsT=n1t[:, dc, sb * P : (sb + 1) * P],
                            rhs=wff_sb[:, dc, db * 512 : (db + 1) * 512],
                            start=(dc == 0),
                            stop=(dc == DC - 1),
                        )
                    nc.vector.tensor_copy(
                        out=y_sb[:, sb, db * 512 : (db + 1) * 512], in_=ps
                    )
                nc.scalar.activation(
                    out=junk,
                    in_=y_sb[:, sb, :],
                    func=AF.Square,
                    accum_out=ss2[:, sb : sb + 1],
                )
            nc.scalar.activation(out=ss2, in_=ss2, func=AF.Ln)
            nc.scalar.activation(out=rs2, in_=ss2, func=AF.Exp, scale=-0.5)

            o_sb = tl.tile([P, 4, D], F32, tag="big")
            out_r = out.rearrange("(sb p) d -> p sb d", p=P)
            for sb in range(4):
                nc.scalar.activation(
                    out=o_sb[:, sb, :],
                    in_=y_sb[:, sb, :],
                    func=AF.Identity if SIM_NO_GELU else AF.Gelu,
                    scale=rs2[:, sb : sb + 1],
                )
                nc.sync.dma_start(out=out_r[:, sb, :], in_=o_sb[:, sb, :])

        w2.release()
        qkv.release()
        misc.release()

    nc.compile()
    return nc


_NC = None


def _get_nc():
    global _NC
    if _NC is None:
        _NC = build_program()
    return _NC


def make_in_maps(x, Wq, Wk, Wv, Wo, Wff):
    import ml_dtypes

    bf16 = np.dtype(ml_dtypes.bfloat16)
    # wo2[t, i*128+p, :] = Wo[i*256 + t*128 + p, :]  (batch-independent:
    # stp block i always holds the same-batch rank-i core's pair-t rows)
    wo2_c = np.empty((2, 4 * 128, 1024), dtype=np.float32)
    for t in range(2):
        for i in range(4):
            r0 = i * 256 + t * 128
            wo2_c[t, i * 128 : (i + 1) * 128, :] = Wo[r0 : r0 + 128, :]
    wo2_c = np.ascontiguousarray(wo2_c).astype(bf16)
    wff_c = np.ascontiguousarray(Wff.astype(np.float32))
    in_maps = []
    for c in range(8):
        b, r = c // 4, c % 4
        mk = np.zeros((128, 8), dtype=np.float32)
        mk[:, b * 4 : (b + 1) * 4] = 1.0
        in_maps.append(
            {
                "xT": np.ascontiguousarray(x[b].T).astype(bf16),
                "wq": np.ascontiguousarray(Wq[:, r * LC : (r + 1) * LC]).astype(bf16),
                "wk": np.ascontiguousarray(Wk[:, r * LC : (r + 1) * LC]).astype(bf16),
                "wv": np.ascontiguousarray(Wv[:, r * LC : (r + 1) * LC]).astype(bf16),
                "wo2": wo2_c,
                "wff": wff_c,
                "msk": mk,
            }
        )
    return in_maps


def run(x, Wq, Wk, Wv, Wo, Wff, trace=False, **spmd_kwargs):
    nc = _get_nc()
    in_maps = make_in_maps(x, Wq, Wk, Wv, Wo, Wff)
    res = run_bass_kernel_spmd(
        nc, in_maps, core_ids=list(range(8)), trace=trace, **spmd_kwargs
    )
    y = np.empty((2, S, D), dtype=np.float32)
    for c in range(8):
        b, r = c // 4, c % 4
        y[b, r * SLICE : (r + 1) * SLICE, :] = res.results[c]["out"]
    return y, res


def kernel(x, Wq, Wk, Wv, Wo, Wff):
    y, _ = run(x, Wq, Wk, Wv, Wo, Wff)
    return y


# revision 5
# speedup vs baseline: 1.0084x; 1.0084x over previous
"""Trainium2 Bass kernel for nn_DecoderOnlyTransformer_10041633538673 (v3).

Sharding: core c = (batch c//4, rank r=c%4) owns heads 4r..4r+3 of its
batch. Head-sharded QKV+attention; one 8-core AllToAll per head-pair
re-shards to sequence-split; Wo/l2norm/Wff/l2norm/gelu on the local
512-query slice.

v3 performance structure:
    - bf16 x/Wq/Wk/Wv (halves startup DMA); per-dc weight DMAs so the
      first projection matmul starts as soon as chunk 0 lands.
    - pair-0 q/k projected first on its own 4 PSUM banks (freed before
      attention); V and pair-1 q/k cycle through the attn-out banks so
      they overlap pair-0's exp stream.
    - attention is Scalar(exp)-paced; matmuls grouped per head (shared
      stationary operand) with walrus ldw-opt eliding redundant weight
      loads; softmax Z via ACT accum_out (cheaper than a DVE reduce).
    - outgoing AllToAll blocks are multiplied by a per-core 0/1 batch
      mask (reads attn-out straight from PSUM, no staging copy);
      receivers add block j + j+4 (exact: one side is zeros), halving
      the Wo contraction to the 4 same-batch blocks.
    - Wo's t=0 half runs inside the second AllToAll's wait window; t=1
      accumulates on top via a DVE add. Gelu/square read Wff's PSUM
      output directly.
"""

import os
import numpy as np

import concourse.bass as bass
import concourse.tile as tile
from concourse import bacc, mybir
from concourse.bass_utils import run_bass_kernel_spmd
from concourse import bass_utils as _bass_utils
from concourse.masks import make_identity

if os.environ.get("KERNEL_LDW_OPT", "0") == "1" and not getattr(_bass_utils, "_ldw_opt_patched", False):
    # consecutive matmuls sharing a stationary operand: walrus's ldw-opt
    # pass elides the redundant weight reloads
    _orig_run_command = _bass_utils.run_command

    def _run_command_ldw(cmd, **kw):
        cmd = [
            "--enable-ldw-opt=true" if c == "--enable-ldw-opt=false" else c
            for c in cmd
        ]
        return _orig_run_command(cmd, **kw)

    _bass_utils.run_command = _run_command_ldw
    _bass_utils._ldw_opt_patched = True

F32 = mybir.dt.float32
F32R = mybir.dt.float32r
BF16 = mybir.dt.bfloat16

P = 128
S = 2048
D = 1024
DH = 64
LC = 256        # local head-cols per core (4 heads x 64)
DC = D // P     # 8 contraction chunks
SBLK = S // P   # 16 seq blocks
SLICE = S // 4  # 512-query slice per core

AF = mybir.ActivationFunctionType
ALU = mybir.AluOpType

SIM_NO_GELU = os.environ.get("KERNEL_SIM_NO_GELU", "0") == "1"


def build_program():
    nc = bacc.Bacc(
        "TRN2",
        target_bir_lowering=False,
        debug=False,
        enable_asserts=False,
        num_devices=8,
    )

    xT = nc.dram_tensor("xT", [D, S], BF16, kind="ExternalInput").ap()
    wq = nc.dram_tensor("wq", [D, LC], BF16, kind="ExternalInput").ap()
    wk = nc.dram_tensor("wk", [D, LC], BF16, kind="ExternalInput").ap()
    wv = nc.dram_tensor("wv", [D, LC], BF16, kind="ExternalInput").ap()
    wo2 = nc.dram_tensor("wo2", [2, 4 * P, D], BF16, kind="ExternalInput").ap()
    wff = nc.dram_tensor("wff", [D, D], F32R, kind="ExternalInput").ap()
    msk = nc.dram_tensor("msk", [P, 8], F32, kind="ExternalInput").ap()
    out = nc.dram_tensor("out", [SLICE, D], F32, kind="ExternalOutput").ap()

    cc_in = [
        nc.dram_tensor(f"cc_in{t}", [8 * P, SLICE], BF16).ap() for t in range(2)
    ]
    cc_out = [
        nc.dram_tensor(f"cc_out{t}", [8 * P, SLICE], BF16).ap() for t in range(2)
    ]

    with tile.TileContext(nc) as tc:
        misc = tc.alloc_tile_pool(name="misc", bufs=1)
        ident = misc.tile([P, P], F32)
        make_identity(nc, ident)
        msk_sb = misc.tile([P, 8], F32, tag="msk")
        nc.sync.dma_start(out=msk_sb, in_=msk)

        qkv = tc.alloc_tile_pool(name="qkv", bufs=1)
        qt = [qkv.tile([P, S], BF16, tag=f"qt{t}", name=f"qt{t}") for t in range(2)]
        kt = [qkv.tile([P, S], BF16, tag=f"kt{t}", name=f"kt{t}") for t in range(2)]
        v_sb = qkv.tile([P, SBLK, LC], BF16, tag="v")

        # weights for the post-attention phases (DMA overlaps phase 1)
        w2 = tc.alloc_tile_pool(name="w2", bufs=1)
        wo2_sb = w2.tile([P, 2, 4, D], BF16, tag="wo2")
        nc.sync.dma_start(out=wo2_sb, in_=wo2.rearrange("t (i p) d -> p t i d", p=P))
        st = w2.tile([P, 2, 8, SLICE], BF16, tag="st")
        stp = w2.tile([P, 2, 4, SLICE], BF16, tag="stp")

        # attn-out PSUM banks (also cycle V + pair-1 q/k projections);
        # closes before the tail so its banks return for Wff
        opp_cm = tc.tile_pool(name="op", bufs=1, space="PSUM")
        opp = opp_cm.__enter__()

        # ---- Phase 1: x^T + projection weights; Q^T/K^T/V.
        with tc.tile_pool(name="xtw", bufs=1) as xtw, tc.tile_pool(
            name="pp1", bufs=1, space="PSUM"
        ) as pp1:
            wq_sb = xtw.tile([P, DC, LC], BF16, tag="wq")
            wk_sb = xtw.tile([P, DC, LC], BF16, tag="wk")
            wv_sb = xtw.tile([P, DC, LC], BF16, tag="wv")
            xt = xtw.tile([P, DC, S], BF16, tag="xt")
            for dc in range(DC):
                nc.sync.dma_start(
                    out=wq_sb[:, dc, :], in_=wq[dc * P : (dc + 1) * P, :]
                )
                nc.sync.dma_start(
                    out=wk_sb[:, dc, :], in_=wk[dc * P : (dc + 1) * P, :]
                )
                nc.sync.dma_start(out=xt[:, dc, :], in_=xT[dc * P : (dc + 1) * P, :])
            nc.sync.dma_start(out=wv_sb, in_=wv.rearrange("(c p) m -> p c m", p=P))

            # pair-0 q/k on dedicated banks (pj*), freed before attention
            for w_sb, dst in ((wq_sb, qt), (wk_sb, kt)):
                pst = [
                    pp1.tile([P, 512], F32, tag=f"pj{i}", name=f"pj{i}")
                    for i in range(4)
                ]
                for dc in range(DC):
                    for sb in range(4):
                        nc.tensor.matmul(
                            pst[sb],
                            lhsT=w_sb[:, dc, 0:P],
                            rhs=xt[:, dc, sb * 512 : (sb + 1) * 512],
                            start=(dc == 0),
                            stop=(dc == DC - 1),
                        )
                for sb in range(4):
                    nc.vector.tensor_copy(
                        out=dst[0][:, sb * 512 : (sb + 1) * 512], in_=pst[sb]
                    )
            # V, then pair-1 q/k, cycling through the op banks
            for sb in range(SBLK):
                ps = opp.tile([P, 512], F32, tag=f"op{sb % 4}", name=f"pv{sb}")
                for dc in range(DC):
                    nc.tensor.matmul(
                        ps[:, 0:LC],
                        lhsT=xt[:, dc, sb * P : (sb + 1) * P],
                        rhs=wv_sb[:, dc, :],
                        start=(dc == 0),
                        stop=(dc == DC - 1),
                    )
                nc.vector.tensor_copy(out=v_sb[:, sb, :], in_=ps[:, 0:LC])
            for w_sb, dst in ((wq_sb, qt), (wk_sb, kt)):
                pst = [
                    opp.tile([P, 512], F32, tag=f"op{i}", name=f"pj1{i}")
                    for i in range(4)
                ]
                for dc in range(DC):
                    for sb in range(4):
                        nc.tensor.matmul(
                            pst[sb],
                            lhsT=w_sb[:, dc, P : 2 * P],
                            rhs=xt[:, dc, sb * 512 : (sb + 1) * 512],
                            start=(dc == 0),
                            stop=(dc == DC - 1),
                        )
                for sb in range(4):
                    nc.vector.tensor_copy(
                        out=dst[1][:, sb * 512 : (sb + 1) * 512], in_=pst[sb]
                    )

        # ---- Phase 2: attention. E = exp(scores/32); Z per head via ACT
        # accum; 1/Z folded into V rows; out^T accumulated in PSUM with
        # the 2 heads of a pair packed into PE column strips.
        with tc.tile_pool(name="att", bufs=10) as att, tc.tile_pool(
            name="sc", bufs=2, space="PSUM"
        ) as scp, tc.tile_pool(name="asml", bufs=4) as asml, tc.tile_pool(
            name="mcp", bufs=2
        ) as mcp:
            for t in range(2):
                o_pp = [
                    opp.tile([P, 512], F32, tag=f"op{qb}", name=f"op{t}{qb}")
                    for qb in range(4)
                ]
                for kb in range(SBLK):
                    kb0 = kb * P
                    e_a = att.tile([P, S], BF16, tag="e", name="e_a")
                    e_b = att.tile([P, S], BF16, tag="e", name="e_b")
                    zp_a = asml.tile([P, 2], F32, tag="zp", name="zp_a")
                    zp_b = asml.tile([P, 2], F32, tag="zp", name="zp_b")
                    # scores: a/b row groups alternate so each LDW
                    # pulls ahead over the other head's in-flight matmul
                    for qh in range(2):
                        sc_a = scp.tile([P, 1024], F32, tag="sc", name="sc_a")
                        sc_b = scp.tile([P, 1024], F32, tag="sc", name="sc_b")
                        for qs in range(2):
                            q0 = qh * 1024 + qs * 512
                            nc.tensor.matmul(
                                sc_a[:, qs * 512 : (qs + 1) * 512],
                                lhsT=kt[t][0:DH, kb0 : kb0 + P],
                                rhs=qt[t][0:DH, q0 : q0 + 512],
                                start=True,
                                stop=True,
                                tile_position=(0, 0),
                                skip_group_check=True,
                            )
                            nc.tensor.matmul(
                                sc_b[:, qs * 512 : (qs + 1) * 512],
                                lhsT=kt[t][DH : 2 * DH, kb0 : kb0 + P],
                                rhs=qt[t][DH : 2 * DH, q0 : q0 + 512],
                                start=True,
                                stop=True,
                                tile_position=(64, 0),
                                skip_group_check=True,
                            )
                        nc.scalar.activation(
                            out=e_a[:, qh * 1024 : (qh + 1) * 1024],
                            in_=sc_a,
                            func=AF.Exp,
                            scale=1.0 / 32.0,
                            accum_out=zp_a[:, qh : qh + 1],
                        )
                        nc.scalar.activation(
                            out=e_b[:, qh * 1024 : (qh + 1) * 1024],
                            in_=sc_b,
                            func=AF.Exp,
                            scale=1.0 / 32.0,
                            accum_out=zp_b[:, qh : qh + 1],
                        )
                    # fold 1/Z into the V rows, then all 8 attn@V matmuls,
                    # grouped per head (shared stationary vp)
                    vps = []
                    for hh, zp in ((0, zp_a), (1, zp_b)):
                        zs = asml.tile([P, 1], F32, tag="zs", name="zs")
                        nc.vector.tensor_add(out=zs, in0=zp[:, 0:1], in1=zp[:, 1:2])
                        zr = asml.tile([P, 1], F32, tag="zr", name="zr")
                        nc.vector.reciprocal(out=zr, in_=zs)
                        vp = asml.tile([P, DH], BF16, tag="vp", name="vp")
                        nc.vector.tensor_scalar_mul(
                            out=vp,
                            in0=v_sb[:, kb, (2 * t + hh) * DH : (2 * t + hh + 1) * DH],
                            scalar1=zr,
                        )
                        vps.append(vp)
                    for qb in range(4):
                        for hh, e_t in ((0, e_a), (1, e_b)):
                            nc.tensor.matmul(
                                o_pp[qb][hh * DH : (hh + 1) * DH, :],
                                lhsT=vps[hh],
                                rhs=e_t[:, qb * 512 : (qb + 1) * 512],
                                start=(kb == 0),
                                stop=(kb == SBLK - 1),
                                tile_position=(0, hh * DH),
                                skip_group_check=True,
                            )
                # stage the AllToAll: mask out cross-batch destination
                # blocks (reads o_pp straight from PSUM)
                mcc = mcp.tile([P, 8, SLICE], BF16, tag="mcc", name=f"mcc{t}")
                for j in range(8):
                    nc.vector.tensor_scalar_mul(
                        out=mcc[:, j, :],
                        in0=o_pp[j % 4],
                        scalar1=msk_sb[:, j : j + 1],
                    )
                    nc.sync.dma_start(
                        out=cc_in[t][j * P : (j + 1) * P, :], in_=mcc[:, j, :]
                    )
                nc.gpsimd.collective_compute(
                    "AllToAll",
                    ALU.bypass,
                    replica_groups=[[0, 1, 2, 3, 4, 5, 6, 7]],
                    ins=[cc_in[t]],
                    outs=[cc_out[t]],
                )

        opp_cm.__exit__(None, None, None)

        # ---- Phase 3: Wo (t=0 half early), l2norm, Wff, l2norm, gelu
        with tc.tile_pool(name="pp2", bufs=2, space="PSUM") as pp2, tc.tile_pool(
            name="tail", bufs=2) as tl, tc.tile_pool(
            name="tsm", bufs=1
        ) as tsm:
            wff_sb = tsm.tile([P, DC, D], F32R, tag="wff")
            nc.sync.dma_start(out=wff_sb, in_=wff.rearrange("(c p) d -> p c d", p=P))
            for t in range(2):
                for i in range(8):
                    nc.sync.dma_start(
                        out=st[:, t, i, :], in_=cc_out[t][i * P : (i + 1) * P, :]
                    )
                for i in range(4):
                    # cross-batch side is exactly zero -> add is exact
                    nc.vector.tensor_add(
                        out=stp[:, t, i, :],
                        in0=st[:, t, i, :],
                        in1=st[:, t, i + 4, :],
                    )

            # t=0 half of Wo runs inside the second AllToAll's wait window
            z0 = tl.tile([P, 4, D], F32, tag="z0")
            for sb in range(4):
                for db in range(2):
                    ps = pp2.tile([P, 512], F32, tag="pp2")
                    for i in range(4):
                        nc.tensor.matmul(
                            ps,
                            lhsT=stp[:, 0, i, sb * P : (sb + 1) * P],
                            rhs=wo2_sb[:, 0, i, db * 512 : (db + 1) * 512],
                            start=(i == 0),
                            stop=(i == 3),
                        )
                    nc.vector.tensor_copy(
                        out=z0[:, sb, db * 512 : (db + 1) * 512], in_=ps
                    )

            z_sb = tl.tile([P, 4, D], F32, tag="big")
            for sb in range(4):
                for db in range(2):
                    ps = pp2.tile([P, 512], F32, tag="pp2")
                    for i in range(4):
                        nc.tensor.matmul(
                            ps,
                            lhsT=stp[:, 1, i, sb * P : (sb + 1) * P],
                            rhs=wo2_sb[:, 1, i, db * 512 : (db + 1) * 512],
                            start=(i == 0),
                            stop=(i == 3),
                        )
                    nc.vector.tensor_add(
                        out=z_sb[:, sb, db * 512 : (db + 1) * 512],
                        in0=ps,
                        in1=z0[:, sb, db * 512 : (db + 1) * 512],
                    )

            junk = tsm.tile([P, D], F32, tag="junk")
            ss1 = tsm.tile([P, 4], F32, tag="ss1")
            rs1 = tsm.tile([P, 4], F32, tag="rs1")
            for sb in range(4):
                nc.scalar.activation(
                    out=junk,
                    in_=z_sb[:, sb, :],
                    func=AF.Square,
                    accum_out=ss1[:, sb : sb + 1],
                )
            # 1/sqrt(ss) = exp(-0.5 * ln(ss)); Ln+Exp share one ACT table set
            nc.scalar.activation(out=ss1, in_=ss1, func=AF.Ln)
            nc.scalar.activation(out=rs1, in_=ss1, func=AF.Exp, scale=-0.5)
            for sb in range(4):
                nc.vector.tensor_scalar_mul(
                    out=z_sb[:, sb, :], in0=z_sb[:, sb, :], scalar1=rs1[:, sb : sb + 1]
                )

            # n1^T via PE transpose
            n1t = tsm.tile([P, DC, SLICE], F32R, tag="n1t")
            for sb in range(4):
                for dc in range(DC):
                    tp = pp2.tile([P, P], F32, tag="tp")
                    nc.tensor.transpose(
                        tp, z_sb[:, sb, dc * P : (dc + 1) * P], ident
                    )
                    nc.vector.tensor_copy(
                        out=n1t[:, dc, sb * P : (sb + 1) * P], in_=tp
                    )

            y_sb = tl.tile([P, 4, D], F32, tag="big")
            ss2 = tsm.tile([P, 4], F32, tag="ss2")
            rs2 = tsm.tile([P, 4], F32, tag="rs2")
            for sb in range(4):
                for db in range(2):
                    ps = pp2.tile([P, 512], F32, tag="pp2")
                    for dc in range(DC):
                        nc.tensor.matmul(
                            ps,
                            lhsT=n1t[:, dc, sb * P : (sb + 1) * P],
                            rhs=wff_sb[:, dc, db * 512 : (db + 1) * 512],
                            start=(dc == 0),
                            stop=(dc == DC - 1),
                        )
                    nc.vector.tensor_copy(
                        out=y_sb[:, sb, db * 512 : (db + 1) * 512], in_=ps
                    )
                nc.scalar.activation(
                    out=junk,
                    in_=y_sb[:, sb, :],
                    func=AF.Square,
                    accum_out=ss2[:, sb : sb + 1],
                )
            nc.scalar.activation(out=ss2, in_=ss2, func=AF.Ln)
            nc.scalar.activation(out=rs2, in_=ss2, func=AF.Exp, scale=-0.5)

            o_sb = tl.tile([P, 4, D], F32, tag="big")
            out_r = out.rearrange("(sb p) d -> p sb d", p=P)
            for sb in range(4):
                nc.scalar.activation(
                    out=o_sb[:, sb, :],
                    in_=y_sb[:, sb, :],
                    func=AF.Identity if SIM_NO_GELU else AF.Gelu,
                    scale=rs2[:, sb : sb + 1],
                )
                nc.sync.dma_start(out=out_r[:, sb, :], in_=o_sb[:, sb, :])

        w2.release()
        qkv.release()
        misc.release()

    nc.compile()
    return nc


_NC = None


def _get_nc():
    global _NC
    if _NC is None:
        _NC = build_program()
    return _NC


def make_in_maps(x, Wq, Wk, Wv, Wo, Wff):
    import ml_dtypes

    bf16 = np.dtype(ml_dtypes.bfloat16)
    # wo2[t, i*128+p, :] = Wo[i*256 + t*128 + p, :]  (batch-independent:
    # stp block i always holds the same-batch rank-i core's pair-t rows)
    wo2_c = np.empty((2, 4 * 128, 1024), dtype=np.float32)
    for t in range(2):
        for i in range(4):
            r0 = i * 256 + t * 128
            wo2_c[t, i * 128 : (i + 1) * 128, :] = Wo[r0 : r0 + 128, :]
    wo2_c = np.ascontiguousarray(wo2_c).astype(bf16)
    wff_c = np.ascontiguousarray(Wff.astype(np.float32))
    in_maps = []
    for c in range(8):
        b, r = c // 4, c % 4
        mk = np.zeros((128, 8), dtype=np.float32)
        mk[:, b * 4 : (b + 1) * 4] = 1.0
        in_maps.append(
            {
                "xT": np.ascontiguousarray(x[b].T).astype(bf16),
                "wq": np.ascontiguousarray(Wq[:, r * LC : (r + 1) * LC]).astype(bf16),
                "wk": np.ascontiguousarray(Wk[:, r * LC : (r + 1) * LC]).astype(bf16),
                "wv": np.ascontiguousarray(Wv[:, r * LC : (r + 1) * LC]).astype(bf16),
                "wo2": wo2_c,
                "wff": wff_c,
                "msk": mk,
            }
        )
    return in_maps


def run(x, Wq, Wk, Wv, Wo, Wff, trace=False, **spmd_kwargs):
    nc = _get_nc()
    in_maps = make_in_maps(x, Wq, Wk, Wv, Wo, Wff)
    res = run_bass_kernel_spmd(
        nc, in_maps, core_ids=list(range(8)), trace=trace, **spmd_kwargs
    )
    y = np.empty((2, S, D), dtype=np.float32)
    for c in range(8):
        b, r = c // 4, c % 4
        y[b, r * SLICE : (r + 1) * SLICE, :] = res.results[c]["out"]
    return y, res


def kernel(x, Wq, Wk, Wv, Wo, Wff):
    y, _ = run(x, Wq, Wk, Wv, Wo, Wff)
    return y
